# revision 10
# baseline (speedup 1.0000x reference)
"""Trainium2 Bass kernel for ContrastiveLoss (N=16384, D=1024, 8 NeuronCores).

Strategy (v3 — squares-only device compute, mixed fp8/fp16, DR+regular PE):
  - Host shards anchor rows across 8 cores (2048 rows each) and marshals
    three streams per core in transposed pair-chunk layout
    [pair, 128, 2, rows]: U (anchors), S = u + v (pos sums), T = u + w
    (neg sums).  Sums-of-pairs make every device op a UNARY square:
    ScalarE (activation Square, dtype-independent, contention-immune)
    carries the fp8 share; VectorE carries an fp16 share at its 2x rate.
    GpSimd does NO compute (its TT ops contend with DVE's SBUF port and
    reduce combined throughput) — it only drives the fast SWDGE DMA
    queue.
  - Dtypes per chunk match the consuming engine: ACT chunks ride as fp8
    (U0, T0-T3), DVE chunks as raw fp16 (S0-S3, U2, U3) plus one early
    fp8 pair (U1).  Queue split: SWDGE (gpsimd, ~350 GB/s) carries the
    fp16 bulk; the two HWDGE queues (sync/scalar, ~100 GB/s each) carry
    the fp8 feed.
  - PE reduces with ones-matmuls into PSUM [96, 2048]: stat T (fp8
    squares) at partitions 0-31 via DoubleRow pair-matmuls (DR dst must
    be partition 0), stats S/U at 32-63/64-95 via regular matmuls.
    MMs issued in expected square-completion order.
  - Extraction: 4x [96, 512] PSUM->SBUF copies split ACT/DVE, one
    strided store of [3, 2048] f32.
  - Host epilogue (f64): q0 = sum u^2, q1 = sum (u+v)^2, q2 = sum (u+w)^2
      uv = (q1 - q0_a - q0_b)/2,  d^2 = ahat2_a + ahat2_b
           - 2 uv/(den_a den_b) + D eps^2, then the margin loss.
"""

import sys

for _p in ("/opt/trn_rl_repo", "/root/.axon_site/_ro/trn_rl_repo"):
    if _p not in sys.path:
        sys.path.append(_p)

import numpy as np
import ml_dtypes

N = 16384  # total rows
D = 1024  # embedding dim
NCORES = 8
RPC = N // NCORES  # rows per core = 2048
KC = D // 128  # k-chunks per core = 8
NPAIR = KC // 2  # chunk pairs = 4
NCG = RPC // 512  # 512-col matmul groups = 4
EPS = 1e-6
MARGIN = 1.0

LAST_RESULT = None
_CACHE = {}

# fp16-in-SBUF pairs (cast-DMA); wire is fp8 everywhere. k=8: S stream.
FP16_PAIRS = [("S", 0), ("S", 1), ("S", 2), ("S", 3)]
ACT_PAIRS = [("U", 0), ("T", 0), ("T", 1), ("T", 2)]
ACT_CHUNKS = [("U", 4)]  # U2a
ACT_PAIRS_TAIL = [("T", 3)]
DVE_PAIRS = [("U", 1), ("S", 0), ("S", 1), ("S", 2), ("U", 3), ("S", 3)]
DVE_CHUNKS = [("U", 5)]  # U2b
# single gp SWDGE queue, consumption order; first pairs halved for latency
# entries: (stream, pair, half) with half in (0, 1, None)
GP_LOADS = [
    ("U", 0, 0), ("U", 1, 0), ("U", 0, 1), ("U", 1, 1),
    ("T", 0, None), ("S", 0, None), ("T", 1, None), ("S", 1, None),
    ("T", 2, None), ("S", 2, None), ("U", 2, None), ("T", 3, None),
    ("U", 3, None), ("S", 3, None),
]

# PE issue order (expected completion order of square pair-units)
PE_ORDER = [
    ("U", 0), ("U", 1), ("T", 0), ("S", 0), ("T", 1), ("S", 1),
    ("T", 2), ("S", 2), ("U", 2), ("U", 3), ("S", 3), ("T", 3),
]
SOFF = {"T": 0, "S": 32, "U": 64}
FIRST_PAIR = {"U": 0, "T": 0, "S": 0}
LAST_PAIR = {"T": 3, "U": 3, "S": 3}
HALVED = {("U", 0), ("U", 1)}  # loaded as two half-pair DMAs (sem target 32)


def ld_target(s, p):
    return 32 if (s, p) in HALVED else 16


def _build_nc():
    import concourse.bass as bass
    import concourse.mybir as mybir

    f32 = mybir.dt.float32
    f16 = mybir.dt.float16
    fp8 = mybir.dt.float8e4
    Sq = mybir.ActivationFunctionType.Square
    mult = mybir.AluOpType.mult
    DR = mybir.MatmulPerfMode.DoubleRow

    nc = bass.Bass()
    # all wire traffic is fp8 pair chunks
    u8p = nc.declare_dram_parameter("u8p", [NPAIR, 128, 2, RPC], fp8, isOutput=False)
    t8p = nc.declare_dram_parameter("t8p", [NPAIR, 128, 2, RPC], fp8, isOutput=False)
    s8p = nc.declare_dram_parameter("s8p", [NPAIR, 128, 2, RPC], fp8, isOutput=False)
    onesp = nc.declare_dram_parameter("onesp", [128, 2, 32], fp8, isOutput=False)
    onesh = nc.declare_dram_parameter("onesh", [128, 32], f16, isOutput=False)
    out = nc.declare_dram_parameter("out", [3, RPC], f32, isOutput=True)

    from contextlib import ExitStack

    with ExitStack() as ctx:
        sb = lambda nm, shape, dt: ctx.enter_context(nc.sbuf_tensor(nm, shape, dt))
        ps_ = lambda nm, shape, dt: ctx.enter_context(nc.psum_tensor(nm, shape, dt))
        sem = lambda nm: ctx.enter_context(nc.semaphore(nm))

        U8 = sb("u8", [128, KC, RPC], fp8)
        T8 = sb("t8", [128, KC, RPC], fp8)
        S16 = sb("s16", [128, KC, RPC], f16)
        # squares: fp16-input ones stay fp16 (fp8 out would drop DVE to 1x)
        QU8 = sb("qu8", [128, KC, RPC], fp8)
        QS16 = sb("qs16", [128, KC, RPC], f16)
        QT = sb("qt", [128, KC, RPC], fp8)
        ONESP = sb("onespb", [128, 2, 32], fp8)
        ONESH = sb("oneshb", [128, 32], f16)
        STATS = sb("stats", [96, RPC], f32)
        PS = ps_("ps", [96, RPC], f32)  # 0-31 T, 32-63 S, 64-95 U

        LD = {s: [sem(f"ld_{s}{p}") for p in range(NPAIR)] for s in "UST"}
        s_ones = sem("s_ones")
        QSEM = {s: [sem(f"q_{s}{p}") for p in range(NPAIR)] for s in "UST"}
        pe_done = sem("pe_done")
        s_ext = sem("s_ext")
        st_sem = sem("st_sem")

        BUF = {"U": U8, "S": S16, "T": T8}
        QB = {"U": QU8, "S": QS16, "T": QT}

        def in_slice(s, p):
            return BUF[s][:, 2 * p : 2 * p + 2, :]

        def dram_slice(s, p):
            return {"U": u8p, "S": s8p, "T": t8p}[s][p]

        def q_slice(s, p):
            return QB[s][:, 2 * p : 2 * p + 2, :]

        def q_chunk(s, c, co):
            return QB[s][:, c, co : co + 512]

        def q_chunk_full(s, c):
            return QB[s][:, c, :]

        def in_chunk_full(s, c):
            return BUF[s][:, c, :]

        def q_is_fp16(s, p):
            return s == "S"

        # ---- loads issued before the block barrier ----
        nc.scalar.dma_start(out=ONESP[:], in_=onesp[:]).then_inc(s_ones, 16)
        nc.scalar.dma_start(out=ONESH[:], in_=onesh[:]).then_inc(s_ones, 16)
        # single SWDGE queue in consumption order; halves inc by 8 each
        for s, p, h in GP_LOADS:
            if h is None:
                nc.gpsimd.dma_start(
                    out=in_slice(s, p), in_=dram_slice(s, p)
                ).then_inc(LD[s][p], 16)
            else:
                nc.gpsimd.dma_start(
                    out=BUF[s][:, 2 * p + h, :], in_=dram_slice(s, p)[:, h, :]
                ).then_inc(LD[s][p], 16)

        block = ctx.enter_context(nc.Block())

        @block.scalar
        def _(scalar):
            for s, p in ACT_PAIRS:
                scalar.wait_ge(LD[s][p], ld_target(s, p))
                nc.scalar.activation(
                    out=q_slice(s, p),
                    in_=in_slice(s, p),
                    func=Sq,
                ).then_inc(QSEM[s][p], 2)
            for s, c in ACT_CHUNKS:
                scalar.wait_ge(LD[s][c // 2], ld_target(s, c // 2))
                nc.scalar.activation(
                    out=q_chunk_full(s, c),
                    in_=in_chunk_full(s, c),
                    func=Sq,
                ).then_inc(QSEM[s][c // 2], 1)
            for s, p in ACT_PAIRS_TAIL:
                scalar.wait_ge(LD[s][p], ld_target(s, p))
                nc.scalar.activation(
                    out=q_slice(s, p),
                    in_=in_slice(s, p),
                    func=Sq,
                ).then_inc(QSEM[s][p], 2)
            for g in (0, 1):
                cs = slice(512 * g, 512 * g + 512)
                scalar.wait_ge(pe_done, 9 + g)
                nc.scalar.copy(out=STATS[0:96, cs], in_=PS[0:96, cs]).then_inc(
                    s_ext, 1
                )

        @block.vector
        def _(vector):
            for s, p in DVE_PAIRS:
                vector.wait_ge(LD[s][p], ld_target(s, p))
                nc.vector.tensor_tensor(
                    out=q_slice(s, p),
                    in0=in_slice(s, p),
                    in1=in_slice(s, p),
                    op=mult,
                ).then_inc(QSEM[s][p], 2)
            for s, c in DVE_CHUNKS:
                vector.wait_ge(LD[s][c // 2], ld_target(s, c // 2))
                nc.vector.tensor_tensor(
                    out=q_chunk_full(s, c),
                    in0=in_chunk_full(s, c),
                    in1=in_chunk_full(s, c),
                    op=mult,
                ).then_inc(QSEM[s][c // 2], 1)
            for g in (2, 3):
                cs = slice(512 * g, 512 * g + 512)
                vector.wait_ge(pe_done, 9 + g)
                nc.vector.tensor_copy(out=STATS[0:96, cs], in_=PS[0:96, cs]).then_inc(
                    s_ext, 1
                )

        @block.tensor
        def _(tensor):
            tensor.wait_ge(s_ones, 32)
            for s, p in PE_ORDER:
                tensor.wait_ge(QSEM[s][p], 2)
                si = SOFF[s]
                start = p == FIRST_PAIR[s]
                stop = p == LAST_PAIR[s]
                if s == "T":
                    for cg in range(NCG):
                        co = 512 * cg
                        mm = nc.tensor.matmul(
                            out=PS[si : si + 32, co : co + 512],
                            lhsT=ONESP[:],
                            rhs=QT[:, 2 * p : 2 * p + 2, co : co + 512],
                            start=start,
                            stop=stop,
                            perf_mode=DR,
                        )
                        if stop:
                            mm.then_inc(pe_done, 1)
                else:
                    lw = ONESH[:] if q_is_fp16(s, p) else ONESP[:, 0, :]
                    if stop:
                        # cg-outer on the closing pair so extraction can
                        # chase col-groups
                        for cg in range(NCG):
                            co = 512 * cg
                            nc.tensor.matmul(
                                out=PS[si : si + 32, co : co + 512],
                                lhsT=lw,
                                rhs=q_chunk(s, 2 * p, co),
                                start=False,
                                stop=False,
                            )
                            nc.tensor.matmul(
                                out=PS[si : si + 32, co : co + 512],
                                lhsT=lw,
                                rhs=q_chunk(s, 2 * p + 1, co),
                                start=False,
                                stop=True,
                            ).then_inc(pe_done, 1)
                    else:
                        for c in (2 * p, 2 * p + 1):
                            for cg in range(NCG):
                                co = 512 * cg
                                nc.tensor.matmul(
                                    out=PS[si : si + 32, co : co + 512],
                                    lhsT=lw,
                                    rhs=q_chunk(s, c, co),
                                    start=start and c == 2 * p,
                                    stop=False,
                                )

        @block.sync
        def _(sync):
            sync.wait_ge(s_ext, 2)
            sync.dma_start(
                out=out[:, 0:1024], in_=STATS[0:96:32, 0:1024]
            ).then_inc(st_sem, 16)
            sync.wait_ge(s_ext, 4)
            sync.dma_start(
                out=out[:, 1024:RPC], in_=STATS[0:96:32, 1024:RPC]
            ).then_inc(st_sem, 16)
            sync.wait_ge(st_sem, 32)

    return nc


def kernel(embeddings, labels, pos_idx, neg_idx):
    global LAST_RESULT
    from concourse.bass_utils import run_bass_kernel_spmd

    emb = np.asarray(embeddings, dtype=np.float32)
    assert emb.shape == (N, D)
    pidx = np.asarray(pos_idx).astype(np.int64)
    nidx = np.asarray(neg_idx).astype(np.int64)

    u8 = emb.astype(ml_dtypes.float8_e4m3)
    s8 = (emb + emb[pidx]).astype(ml_dtypes.float8_e4m3)
    t8 = (emb + emb[nidx]).astype(ml_dtypes.float8_e4m3)
    onesp = np.ones((128, 2, 32), dtype=ml_dtypes.float8_e4m3)
    onesh = np.ones((128, 32), dtype=np.float16)

    def tchunks(rows):
        t = np.ascontiguousarray(rows.T).reshape(KC, 128, RPC)
        return np.ascontiguousarray(
            t.reshape(NPAIR, 2, 128, RPC).transpose(0, 2, 1, 3)
        )

    in_maps = []
    for i in range(NCORES):
        sl = slice(i * RPC, (i + 1) * RPC)
        in_maps.append(
            {
                "u8p": tchunks(u8[sl]),
                "s8p": tchunks(s8[sl]),
                "t8p": tchunks(t8[sl]),
                "onesp": onesp,
                "onesh": onesh,
            }
        )

    nc = _CACHE.get("nc")
    if nc is None:
        nc = _build_nc()
        _CACHE["nc"] = nc

    res = run_bass_kernel_spmd(nc, in_maps, list(range(NCORES)))
    LAST_RESULT = res

    def decode(k):
        return np.concatenate(
            [res.results[i]["out"][k] for i in range(NCORES)]
        ).astype(np.float64)

    # psum rows: 0-31 T, 32-63 S, 64-95 U
    q2 = decode(0)  # sum (u+w)^2
    q1 = decode(1)  # sum (u+v)^2
    q0 = decode(2)  # sum u^2

    den = np.maximum(np.sqrt(q0), EPS)
    ahat2 = q0 / (den * den)

    def dist(idx, q):
        dot = (q - q0 - q0[idx]) / 2.0
        S = ahat2 + ahat2[idx] - 2.0 * dot / (den * den[idx]) + D * EPS * EPS
        return np.sqrt(np.maximum(S, 0.0)) + EPS

    d_pos = dist(pidx, q1)
    d_neg = dist(nidx, q2)
    pos_loss = d_pos * d_pos
    neg_loss = np.maximum(MARGIN - d_neg, EPS) ** 2
    total = pos_loss.sum() + neg_loss.sum()
    return np.array(total / (2.0 * N), dtype=np.float32)


# revision 12
# speedup vs baseline: 1.1303x; 1.1303x over previous
"""Trainium2 Bass kernel for ContrastiveLoss (N=16384, D=1024, 8 NeuronCores).

Strategy (v3 — squares-only device compute, mixed fp8/fp16, DR+regular PE):
  - Host shards anchor rows across 8 cores (2048 rows each) and marshals
    three streams per core in transposed pair-chunk layout
    [pair, 128, 2, rows]: U (anchors), S = u + v (pos sums), T = u + w
    (neg sums).  Sums-of-pairs make every device op a UNARY square:
    ScalarE (activation Square, dtype-independent, contention-immune)
    carries the fp8 share; VectorE carries an fp16 share at its 2x rate.
    GpSimd does NO compute (its TT ops contend with DVE's SBUF port and
    reduce combined throughput) — it only drives the fast SWDGE DMA
    queue.
  - Dtypes per chunk match the consuming engine: ACT chunks ride as fp8
    (U0, T0-T3), DVE chunks as raw fp16 (S0-S3, U2, U3) plus one early
    fp8 pair (U1).  Queue split: SWDGE (gpsimd, ~350 GB/s) carries the
    fp16 bulk; the two HWDGE queues (sync/scalar, ~100 GB/s each) carry
    the fp8 feed.
  - PE reduces with ones-matmuls into PSUM [96, 2048]: stat T (fp8
    squares) at partitions 0-31 via DoubleRow pair-matmuls (DR dst must
    be partition 0), stats S/U at 32-63/64-95 via regular matmuls.
    MMs issued in expected square-completion order.
  - Extraction: 4x [96, 512] PSUM->SBUF copies split ACT/DVE, one
    strided store of [3, 2048] f32.
  - Host epilogue (f64): q0 = sum u^2, q1 = sum (u+v)^2, q2 = sum (u+w)^2
      uv = (q1 - q0_a - q0_b)/2,  d^2 = ahat2_a + ahat2_b
           - 2 uv/(den_a den_b) + D eps^2, then the margin loss.
"""

import sys

for _p in ("/opt/trn_rl_repo", "/root/.axon_site/_ro/trn_rl_repo"):
    if _p not in sys.path:
        sys.path.append(_p)

import numpy as np
import ml_dtypes

N = 16384  # total rows
D = 1024  # embedding dim
NCORES = 8
RPC = N // NCORES  # rows per core = 2048
KC = D // 128  # k-chunks per core = 8
NPAIR = KC // 2  # chunk pairs = 4
NCG = RPC // 512  # 512-col matmul groups = 4
EPS = 1e-6
MARGIN = 1.0

LAST_RESULT = None
_CACHE = {}

# fp16-in-SBUF pairs (cast-DMA); wire is fp8 everywhere. k=8: S stream.
FP16_PAIRS = [("S", 0), ("S", 1), ("S", 2), ("S", 3)]
# engine unit lists: ("pair", s, p) or ("chunk", s, c)
ACT_UNITS = [
    ("chunk", "U", 0), ("chunk", "U", 1), ("pair", "T", 0), ("pair", "T", 1),
    ("pair", "T", 2), ("chunk", "U", 4), ("pair", "T", 3),
]
DVE_UNITS = [
    ("chunk", "U", 2), ("chunk", "U", 3), ("pair", "S", 0), ("pair", "S", 1),
    ("pair", "S", 2), ("chunk", "U", 5), ("pair", "U", 3), ("pair", "S", 3),
]
# single gp SWDGE queue, consumption order; first pairs halved for latency
# entries: (stream, pair, half) with half in (0, 1, None)
GP_LOADS = [
    ("U", 0, 0), ("U", 1, 0), ("U", 0, 1), ("U", 1, 1),
    ("T", 0, None), ("S", 0, None), ("T", 1, None), ("S", 1, None),
    ("T", 2, None), ("U", 2, None), ("S", 2, None), ("U", 3, None),
    ("S", 3, None), ("T", 3, None),
]

# PE issue order (expected completion order of square pair-units)
PE_ORDER = [
    ("U", 0), ("U", 1), ("T", 0), ("S", 0), ("T", 1), ("S", 1),
    ("T", 2), ("U", 2), ("S", 2), ("U", 3), ("S", 3), ("T", 3),
]
SOFF = {"T": 0, "S": 32, "U": 64}
FIRST_PAIR = {"U": 0, "T": 0, "S": 0}
LAST_PAIR = {"T": 3, "U": 3, "S": 3}
HALVED = {("U", 0), ("U", 1)}  # loaded as two half-pair DMAs (sem target 32)


def ld_target(s, p):
    return 32 if (s, p) in HALVED else 16


def _build_nc():
    import concourse.bass as bass
    import concourse.mybir as mybir

    f32 = mybir.dt.float32
    f16 = mybir.dt.float16
    fp8 = mybir.dt.float8e4
    Sq = mybir.ActivationFunctionType.Square
    mult = mybir.AluOpType.mult
    DR = mybir.MatmulPerfMode.DoubleRow

    nc = bass.Bass()
    # all wire traffic is fp8 pair chunks
    u8p = nc.declare_dram_parameter("u8p", [NPAIR, 128, 2, RPC], fp8, isOutput=False)
    t8p = nc.declare_dram_parameter("t8p", [NPAIR, 128, 2, RPC], fp8, isOutput=False)
    s8p = nc.declare_dram_parameter("s8p", [NPAIR, 128, 2, RPC], fp8, isOutput=False)
    out = nc.declare_dram_parameter("out", [3, RPC], f32, isOutput=True)

    from contextlib import ExitStack

    with ExitStack() as ctx:
        sb = lambda nm, shape, dt: ctx.enter_context(nc.sbuf_tensor(nm, shape, dt))
        ps_ = lambda nm, shape, dt: ctx.enter_context(nc.psum_tensor(nm, shape, dt))
        sem = lambda nm: ctx.enter_context(nc.semaphore(nm))

        U8 = sb("u8", [128, KC, RPC], fp8)
        T8 = sb("t8", [128, KC, RPC], fp8)
        S16 = sb("s16", [128, KC, RPC], f16)
        # squares: fp16-input ones stay fp16 (fp8 out would drop DVE to 1x)
        QU8 = sb("qu8", [128, KC, RPC], fp8)
        QS16 = sb("qs16", [128, KC, RPC], f16)
        QT = sb("qt", [128, KC, RPC], fp8)
        ONESP = sb("onespb", [128, 2, 32], fp8)
        ONESH = sb("oneshb", [128, 32], f16)
        STATS = sb("stats", [96, RPC], f32)
        PS = ps_("ps", [96, RPC], f32)  # 0-31 T, 32-63 S, 64-95 U

        LD = {s: [sem(f"ld_{s}{p}") for p in range(NPAIR)] for s in "UST"}
        s_ones = sem("s_ones")
        QSEM = {s: [sem(f"q_{s}{p}") for p in range(NPAIR)] for s in "UST"}
        pe_done = sem("pe_done")
        s_ext = sem("s_ext")
        st_sem = sem("st_sem")

        BUF = {"U": U8, "S": S16, "T": T8}
        QB = {"U": QU8, "S": QS16, "T": QT}

        def in_slice(s, p):
            return BUF[s][:, 2 * p : 2 * p + 2, :]

        def dram_slice(s, p):
            return {"U": u8p, "S": s8p, "T": t8p}[s][p]

        def q_slice(s, p):
            return QB[s][:, 2 * p : 2 * p + 2, :]

        def q_chunk(s, c, co):
            return QB[s][:, c, co : co + 512]

        def q_chunk_full(s, c):
            return QB[s][:, c, :]

        def in_chunk_full(s, c):
            return BUF[s][:, c, :]

        def q_is_fp16(s, p):
            return s == "S"

        # ---- ones built on-device; loads issued before the block barrier ----
        nc.gpsimd.memset(ONESP[:], 1.0).then_inc(s_ones, 1)
        nc.gpsimd.memset(ONESH[:], 1.0).then_inc(s_ones, 1)
        # single SWDGE queue in consumption order
        for s, p, h in GP_LOADS:
            if h is None:
                nc.gpsimd.dma_start(
                    out=in_slice(s, p), in_=dram_slice(s, p)
                ).then_inc(LD[s][p], 16)
            else:
                nc.gpsimd.dma_start(
                    out=BUF[s][:, 2 * p + h, :], in_=dram_slice(s, p)[:, h, :]
                ).then_inc(LD[s][p], 16)

        block = ctx.enter_context(nc.Block())

        def chunk_ld_wait(eng, s, c):
            # halves of halved pairs land in order h0 then h1
            p = c // 2
            if (s, p) in HALVED:
                eng.wait_ge(LD[s][p], 16 if c % 2 == 0 else 32)
            else:
                eng.wait_ge(LD[s][p], 16)

        @block.scalar
        def _(scalar):
            for kind, s, i in ACT_UNITS:
                if kind == "pair":
                    scalar.wait_ge(LD[s][i], ld_target(s, i))
                    nc.scalar.activation(
                        out=q_slice(s, i), in_=in_slice(s, i), func=Sq
                    ).then_inc(QSEM[s][i], 2)
                else:
                    chunk_ld_wait(scalar, s, i)
                    nc.scalar.activation(
                        out=q_chunk_full(s, i), in_=in_chunk_full(s, i), func=Sq
                    ).then_inc(QSEM[s][i // 2], 1)
            for g in (0, 1):
                cs = slice(512 * g, 512 * g + 512)
                scalar.wait_ge(pe_done, 9 + g)
                nc.scalar.copy(out=STATS[0:96, cs], in_=PS[0:96, cs]).then_inc(
                    s_ext, 1
                )

        @block.vector
        def _(vector):
            for kind, s, i in DVE_UNITS:
                if kind == "pair":
                    vector.wait_ge(LD[s][i], ld_target(s, i))
                    nc.vector.tensor_tensor(
                        out=q_slice(s, i),
                        in0=in_slice(s, i),
                        in1=in_slice(s, i),
                        op=mult,
                    ).then_inc(QSEM[s][i], 2)
                else:
                    chunk_ld_wait(vector, s, i)
                    nc.vector.tensor_tensor(
                        out=q_chunk_full(s, i),
                        in0=in_chunk_full(s, i),
                        in1=in_chunk_full(s, i),
                        op=mult,
                    ).then_inc(QSEM[s][i // 2], 1)
            for g in (2, 3):
                cs = slice(512 * g, 512 * g + 512)
                vector.wait_ge(pe_done, 9 + g)
                nc.vector.tensor_copy(out=STATS[0:96, cs], in_=PS[0:96, cs]).then_inc(
                    s_ext, 1
                )

        DUMMY = sb("warmup_rhs", [128, 512], fp8)

        @block.tensor
        def _(tensor):
            tensor.wait_ge(s_ones, 2)
            # warm-up matmuls: keep PE busy/unthrottled until real work;
            # garbage results are overwritten by each region's start=True MM
            for i in range(20):
                nc.tensor.matmul(
                    out=PS[0:32, 0:512],
                    lhsT=ONESP[:, 0, :],
                    rhs=DUMMY[:],
                    start=True,
                    stop=True,
                    skip_group_check=True,
                )
            for s, p in PE_ORDER:
                tensor.wait_ge(QSEM[s][p], 2)
                si = SOFF[s]
                start = p == FIRST_PAIR[s]
                stop = p == LAST_PAIR[s]
                if s == "T":
                    for cg in range(NCG):
                        co = 512 * cg
                        mm = nc.tensor.matmul(
                            out=PS[si : si + 32, co : co + 512],
                            lhsT=ONESP[:],
                            rhs=QT[:, 2 * p : 2 * p + 2, co : co + 512],
                            start=start,
                            stop=stop,
                            perf_mode=DR,
                        )
                        if stop:
                            mm.then_inc(pe_done, 1)
                else:
                    lw = ONESH[:] if q_is_fp16(s, p) else ONESP[:, 0, :]
                    if stop:
                        # cg-outer on the closing pair so extraction can
                        # chase col-groups
                        for cg in range(NCG):
                            co = 512 * cg
                            nc.tensor.matmul(
                                out=PS[si : si + 32, co : co + 512],
                                lhsT=lw,
                                rhs=q_chunk(s, 2 * p, co),
                                start=False,
                                stop=False,
                            )
                            nc.tensor.matmul(
                                out=PS[si : si + 32, co : co + 512],
                                lhsT=lw,
                                rhs=q_chunk(s, 2 * p + 1, co),
                                start=False,
                                stop=True,
                            ).then_inc(pe_done, 1)
                    else:
                        for c in (2 * p, 2 * p + 1):
                            for cg in range(NCG):
                                co = 512 * cg
                                nc.tensor.matmul(
                                    out=PS[si : si + 32, co : co + 512],
                                    lhsT=lw,
                                    rhs=q_chunk(s, c, co),
                                    start=start and c == 2 * p,
                                    stop=False,
                                )

        @block.sync
        def _(sync):
            sync.wait_ge(s_ext, 2)
            sync.dma_start(
                out=out[:, 0:1024], in_=STATS[0:96:32, 0:1024]
            ).then_inc(st_sem, 16)
            sync.wait_ge(s_ext, 4)
            sync.dma_start(
                out=out[:, 1024:RPC], in_=STATS[0:96:32, 1024:RPC]
            ).then_inc(st_sem, 16)
            sync.wait_ge(st_sem, 32)

    return nc


def kernel(embeddings, labels, pos_idx, neg_idx):
    global LAST_RESULT
    from concourse.bass_utils import run_bass_kernel_spmd

    emb = np.asarray(embeddings, dtype=np.float32)
    assert emb.shape == (N, D)
    pidx = np.asarray(pos_idx).astype(np.int64)
    nidx = np.asarray(neg_idx).astype(np.int64)

    u8 = emb.astype(ml_dtypes.float8_e4m3)
    s8 = (emb + emb[pidx]).astype(ml_dtypes.float8_e4m3)
    t8 = (emb + emb[nidx]).astype(ml_dtypes.float8_e4m3)

    def tchunks(rows):
        t = np.ascontiguousarray(rows.T).reshape(KC, 128, RPC)
        return np.ascontiguousarray(
            t.reshape(NPAIR, 2, 128, RPC).transpose(0, 2, 1, 3)
        )

    in_maps = []
    for i in range(NCORES):
        sl = slice(i * RPC, (i + 1) * RPC)
        in_maps.append(
            {
                "u8p": tchunks(u8[sl]),
                "s8p": tchunks(s8[sl]),
                "t8p": tchunks(t8[sl]),
            }
        )

    nc = _CACHE.get("nc")
    if nc is None:
        nc = _build_nc()
        _CACHE["nc"] = nc

    res = run_bass_kernel_spmd(nc, in_maps, list(range(NCORES)))
    LAST_RESULT = res

    def decode(k):
        return np.concatenate(
            [res.results[i]["out"][k] for i in range(NCORES)]
        ).astype(np.float64)

    # psum rows: 0-31 T, 32-63 S, 64-95 U
    q2 = decode(0)  # sum (u+w)^2
    q1 = decode(1)  # sum (u+v)^2
    q0 = decode(2)  # sum u^2

    den = np.maximum(np.sqrt(q0), EPS)
    ahat2 = q0 / (den * den)

    def dist(idx, q):
        dot = (q - q0 - q0[idx]) / 2.0
        S = ahat2 + ahat2[idx] - 2.0 * dot / (den * den[idx]) + D * EPS * EPS
        return np.sqrt(np.maximum(S, 0.0)) + EPS

    d_pos = dist(pidx, q1)
    d_neg = dist(nidx, q2)
    pos_loss = d_pos * d_pos
    neg_loss = np.maximum(MARGIN - d_neg, EPS) ** 2
    total = pos_loss.sum() + neg_loss.sum()
    return np.array(total / (2.0 * N), dtype=np.float32)


# revision 13
# speedup vs baseline: 1.1897x; 1.0526x over previous
"""Trainium2 Bass kernel for ContrastiveLoss (N=16384, D=1024, 8 NeuronCores).

Strategy (v3 — squares-only device compute, mixed fp8/fp16, DR+regular PE):
  - Host shards anchor rows across 8 cores (2048 rows each) and marshals
    three streams per core in transposed pair-chunk layout
    [pair, 128, 2, rows]: U (anchors), S = u + v (pos sums), T = u + w
    (neg sums).  Sums-of-pairs make every device op a UNARY square:
    ScalarE (activation Square, dtype-independent, contention-immune)
    carries the fp8 share; VectorE carries an fp16 share at its 2x rate.
    GpSimd does NO compute (its TT ops contend with DVE's SBUF port and
    reduce combined throughput) — it only drives the fast SWDGE DMA
    queue.
  - Dtypes per chunk match the consuming engine: ACT chunks ride as fp8
    (U0, T0-T3), DVE chunks as raw fp16 (S0-S3, U2, U3) plus one early
    fp8 pair (U1).  Queue split: SWDGE (gpsimd, ~350 GB/s) carries the
    fp16 bulk; the two HWDGE queues (sync/scalar, ~100 GB/s each) carry
    the fp8 feed.
  - PE reduces with ones-matmuls into PSUM [96, 2048]: stat T (fp8
    squares) at partitions 0-31 via DoubleRow pair-matmuls (DR dst must
    be partition 0), stats S/U at 32-63/64-95 via regular matmuls.
    MMs issued in expected square-completion order.
  - Extraction: 4x [96, 512] PSUM->SBUF copies split ACT/DVE, one
    strided store of [3, 2048] f32.
  - Host epilogue (f64): q0 = sum u^2, q1 = sum (u+v)^2, q2 = sum (u+w)^2
      uv = (q1 - q0_a - q0_b)/2,  d^2 = ahat2_a + ahat2_b
           - 2 uv/(den_a den_b) + D eps^2, then the margin loss.
"""

import sys

for _p in ("/opt/trn_rl_repo", "/root/.axon_site/_ro/trn_rl_repo"):
    if _p not in sys.path:
        sys.path.append(_p)

import numpy as np
import ml_dtypes

N = 16384  # total rows
D = 1024  # embedding dim
NCORES = 8
RPC = N // NCORES  # rows per core = 2048
KC = D // 128  # k-chunks per core = 8
NPAIR = KC // 2  # chunk pairs = 4
NCG = RPC // 512  # 512-col matmul groups = 4
EPS = 1e-6
MARGIN = 1.0

LAST_RESULT = None
_CACHE = {}

# fp16-in-SBUF pairs (cast-DMA); wire is fp8 everywhere. k=8: S stream.
FP16_PAIRS = [("S", 0), ("S", 1), ("S", 2), ("S", 3)]
# engine unit lists: ("pair", s, p) or ("chunk", s, c)
ACT_UNITS = [
    ("chunk", "U", 0), ("chunk", "U", 1), ("pair", "T", 0), ("pair", "T", 1),
    ("pair", "T", 2), ("pair", "T", 3), ("chunk", "U", 6),
]
DVE_UNITS = [
    ("chunk", "U", 2), ("chunk", "U", 3), ("pair", "S", 0), ("pair", "S", 1),
    ("pair", "S", 2), ("chunk", "U", 4), ("chunk", "U", 5), ("pair", "S", 3),
    ("chunk", "U", 7),
]
# gp SWDGE queue carries U + S (+T0); T1-T3 ride the two HWDGE queues
# entries: (stream, pair, half) with half in (0, 1, None)
GP_LOADS = [
    ("U", 0, 0), ("U", 1, 0), ("U", 0, 1), ("U", 1, 1),
    ("T", 0, None), ("S", 0, None), ("S", 1, None), ("S", 2, None),
    ("U", 2, None), ("S", 3, None), ("U", 3, None),
]
SYNC_LOADS = [("T", 1), ("T", 3)]
SCALAR_LOADS = [("T", 2)]

# PE issue order (expected completion order of square pair-units)
PE_ORDER = [
    ("U", 0), ("U", 1), ("S", 0), ("T", 0), ("S", 1), ("T", 1),
    ("S", 2), ("T", 2), ("U", 2), ("S", 3), ("T", 3), ("U", 3),
]
SOFF = {"T": 0, "S": 32, "U": 64}
FIRST_PAIR = {"U": 0, "T": 0, "S": 0}
LAST_PAIR = {"T": 3, "U": 3, "S": 3}
HALVED = {("U", 0), ("U", 1)}  # loaded as two half-pair DMAs (sem target 32)


def ld_target(s, p):
    return 32 if (s, p) in HALVED else 16


def _build_nc():
    import concourse.bass as bass
    import concourse.mybir as mybir

    f32 = mybir.dt.float32
    f16 = mybir.dt.float16
    fp8 = mybir.dt.float8e4
    Sq = mybir.ActivationFunctionType.Square
    mult = mybir.AluOpType.mult
    DR = mybir.MatmulPerfMode.DoubleRow

    nc = bass.Bass()
    # all wire traffic is fp8 pair chunks
    u8p = nc.declare_dram_parameter("u8p", [NPAIR, 128, 2, RPC], fp8, isOutput=False)
    t8p = nc.declare_dram_parameter("t8p", [NPAIR, 128, 2, RPC], fp8, isOutput=False)
    s8p = nc.declare_dram_parameter("s8p", [NPAIR, 128, 2, RPC], fp8, isOutput=False)
    out = nc.declare_dram_parameter("out", [3, RPC], f32, isOutput=True)

    from contextlib import ExitStack

    with ExitStack() as ctx:
        sb = lambda nm, shape, dt: ctx.enter_context(nc.sbuf_tensor(nm, shape, dt))
        ps_ = lambda nm, shape, dt: ctx.enter_context(nc.psum_tensor(nm, shape, dt))
        sem = lambda nm: ctx.enter_context(nc.semaphore(nm))

        U8 = sb("u8", [128, KC, RPC], fp8)
        T8 = sb("t8", [128, KC, RPC], fp8)
        S16 = sb("s16", [128, KC, RPC], f16)
        # squares: U/S stored fp16 (kills the e4m3 squares bias; fp8-in
        # fp16-out costs DVE nothing — it is 1x either way). T must stay
        # fp8 for DoubleRow; its bias is clamped away on the neg side.
        QU16 = sb("qu16", [128, KC, RPC], f16)
        QS16 = sb("qs16", [128, KC, RPC], f16)
        QT = sb("qt", [128, KC, RPC], fp8)
        ONESP = sb("onespb", [128, 2, 32], fp8)
        ONESH = sb("oneshb", [128, 32], f16)
        STATS = sb("stats", [96, RPC], f32)
        PS = ps_("ps", [96, RPC], f32)  # 0-31 T, 32-63 S, 64-95 U

        LD = {s: [sem(f"ld_{s}{p}") for p in range(NPAIR)] for s in "UST"}
        s_ones = sem("s_ones")
        QSEM = {s: [sem(f"q_{s}{p}") for p in range(NPAIR)] for s in "UST"}
        pe_done = sem("pe_done")
        s_ext = sem("s_ext")
        st_sem = sem("st_sem")

        BUF = {"U": U8, "S": S16, "T": T8}
        QB = {"U": QU16, "S": QS16, "T": QT}

        def in_slice(s, p):
            return BUF[s][:, 2 * p : 2 * p + 2, :]

        def dram_slice(s, p):
            return {"U": u8p, "S": s8p, "T": t8p}[s][p]

        def q_slice(s, p):
            return QB[s][:, 2 * p : 2 * p + 2, :]

        def q_chunk(s, c, co):
            return QB[s][:, c, co : co + 512]

        def q_chunk_full(s, c):
            return QB[s][:, c, :]

        def in_chunk_full(s, c):
            return BUF[s][:, c, :]

        def q_is_fp16(s, p):
            return s in ("S", "U")

        # ---- ones built on-device; loads issued before the block barrier ----
        nc.gpsimd.memset(ONESP[:], 1.0).then_inc(s_ones, 1)
        nc.gpsimd.memset(ONESH[:], 1.0).then_inc(s_ones, 1)
        for s, p in SYNC_LOADS:
            nc.sync.dma_start(out=in_slice(s, p), in_=dram_slice(s, p)).then_inc(
                LD[s][p], 16
            )
        for s, p in SCALAR_LOADS:
            nc.scalar.dma_start(out=in_slice(s, p), in_=dram_slice(s, p)).then_inc(
                LD[s][p], 16
            )
        # SWDGE queue in consumption order
        for s, p, h in GP_LOADS:
            if h is None:
                nc.gpsimd.dma_start(
                    out=in_slice(s, p), in_=dram_slice(s, p)
                ).then_inc(LD[s][p], 16)
            else:
                nc.gpsimd.dma_start(
                    out=BUF[s][:, 2 * p + h, :], in_=dram_slice(s, p)[:, h, :]
                ).then_inc(LD[s][p], 16)

        block = ctx.enter_context(nc.Block())

        def chunk_ld_wait(eng, s, c):
            # halves of halved pairs land in order h0 then h1
            p = c // 2
            if (s, p) in HALVED:
                eng.wait_ge(LD[s][p], 16 if c % 2 == 0 else 32)
            else:
                eng.wait_ge(LD[s][p], 16)

        @block.scalar
        def _(scalar):
            for kind, s, i in ACT_UNITS:
                if kind == "pair":
                    scalar.wait_ge(LD[s][i], ld_target(s, i))
                    nc.scalar.activation(
                        out=q_slice(s, i), in_=in_slice(s, i), func=Sq
                    ).then_inc(QSEM[s][i], 2)
                else:
                    chunk_ld_wait(scalar, s, i)
                    nc.scalar.activation(
                        out=q_chunk_full(s, i), in_=in_chunk_full(s, i), func=Sq
                    ).then_inc(QSEM[s][i // 2], 1)
            for g in (0, 1):
                cs = slice(512 * g, 512 * g + 512)
                scalar.wait_ge(pe_done, 9 + g)
                nc.scalar.copy(out=STATS[0:96, cs], in_=PS[0:96, cs]).then_inc(
                    s_ext, 1
                )

        @block.vector
        def _(vector):
            for kind, s, i in DVE_UNITS:
                if kind == "pair":
                    vector.wait_ge(LD[s][i], ld_target(s, i))
                    nc.vector.tensor_tensor(
                        out=q_slice(s, i),
                        in0=in_slice(s, i),
                        in1=in_slice(s, i),
                        op=mult,
                    ).then_inc(QSEM[s][i], 2)
                else:
                    chunk_ld_wait(vector, s, i)
                    nc.vector.tensor_tensor(
                        out=q_chunk_full(s, i),
                        in0=in_chunk_full(s, i),
                        in1=in_chunk_full(s, i),
                        op=mult,
                    ).then_inc(QSEM[s][i // 2], 1)
            for g in (2, 3):
                cs = slice(512 * g, 512 * g + 512)
                vector.wait_ge(pe_done, 9 + g)
                nc.vector.tensor_copy(out=STATS[0:96, cs], in_=PS[0:96, cs]).then_inc(
                    s_ext, 1
                )

        DUMMY = sb("warmup_rhs", [128, 512], fp8)

        @block.tensor
        def _(tensor):
            tensor.wait_ge(s_ones, 2)
            # warm-up matmuls: keep PE busy/unthrottled until real work;
            # garbage results are overwritten by each region's start=True MM
            for i in range(20):
                nc.tensor.matmul(
                    out=PS[0:32, 0:512],
                    lhsT=ONESP[:, 0, :],
                    rhs=DUMMY[:],
                    start=True,
                    stop=True,
                    skip_group_check=True,
                )
            for s, p in PE_ORDER:
                tensor.wait_ge(QSEM[s][p], 2)
                si = SOFF[s]
                start = p == FIRST_PAIR[s]
                stop = p == LAST_PAIR[s]
                if s == "T":
                    for cg in range(NCG):
                        co = 512 * cg
                        mm = nc.tensor.matmul(
                            out=PS[si : si + 32, co : co + 512],
                            lhsT=ONESP[:],
                            rhs=QT[:, 2 * p : 2 * p + 2, co : co + 512],
                            start=start,
                            stop=stop,
                            perf_mode=DR,
                        )
                        if stop:
                            mm.then_inc(pe_done, 1)
                else:
                    lw = ONESH[:] if q_is_fp16(s, p) else ONESP[:, 0, :]
                    if stop:
                        # cg-outer on the closing pair so extraction can
                        # chase col-groups
                        for cg in range(NCG):
                            co = 512 * cg
                            nc.tensor.matmul(
                                out=PS[si : si + 32, co : co + 512],
                                lhsT=lw,
                                rhs=q_chunk(s, 2 * p, co),
                                start=False,
                                stop=False,
                            )
                            nc.tensor.matmul(
                                out=PS[si : si + 32, co : co + 512],
                                lhsT=lw,
                                rhs=q_chunk(s, 2 * p + 1, co),
                                start=False,
                                stop=True,
                            ).then_inc(pe_done, 1)
                    else:
                        for c in (2 * p, 2 * p + 1):
                            for cg in range(NCG):
                                co = 512 * cg
                                nc.tensor.matmul(
                                    out=PS[si : si + 32, co : co + 512],
                                    lhsT=lw,
                                    rhs=q_chunk(s, c, co),
                                    start=start and c == 2 * p,
                                    stop=False,
                                )

        @block.sync
        def _(sync):
            sync.wait_ge(s_ext, 2)
            sync.dma_start(
                out=out[:, 0:1024], in_=STATS[0:96:32, 0:1024]
            ).then_inc(st_sem, 16)
            sync.wait_ge(s_ext, 4)
            sync.dma_start(
                out=out[:, 1024:RPC], in_=STATS[0:96:32, 1024:RPC]
            ).then_inc(st_sem, 16)
            sync.wait_ge(st_sem, 32)

    return nc


def kernel(embeddings, labels, pos_idx, neg_idx):
    global LAST_RESULT
    from concourse.bass_utils import run_bass_kernel_spmd

    emb = np.asarray(embeddings, dtype=np.float32)
    assert emb.shape == (N, D)
    pidx = np.asarray(pos_idx).astype(np.int64)
    nidx = np.asarray(neg_idx).astype(np.int64)

    u8 = emb.astype(ml_dtypes.float8_e4m3)
    s8 = (emb + emb[pidx]).astype(ml_dtypes.float8_e4m3)
    t8 = (emb + emb[nidx]).astype(ml_dtypes.float8_e4m3)

    def tchunks(rows):
        t = np.ascontiguousarray(rows.T).reshape(KC, 128, RPC)
        return np.ascontiguousarray(
            t.reshape(NPAIR, 2, 128, RPC).transpose(0, 2, 1, 3)
        )

    in_maps = []
    for i in range(NCORES):
        sl = slice(i * RPC, (i + 1) * RPC)
        in_maps.append(
            {
                "u8p": tchunks(u8[sl]),
                "s8p": tchunks(s8[sl]),
                "t8p": tchunks(t8[sl]),
            }
        )

    nc = _CACHE.get("nc")
    if nc is None:
        nc = _build_nc()
        _CACHE["nc"] = nc

    res = run_bass_kernel_spmd(nc, in_maps, list(range(NCORES)))
    LAST_RESULT = res

    def decode(k):
        return np.concatenate(
            [res.results[i]["out"][k] for i in range(NCORES)]
        ).astype(np.float64)

    # psum rows: 0-31 T, 32-63 S, 64-95 U
    q2 = decode(0)  # sum (u+w)^2
    q1 = decode(1)  # sum (u+v)^2
    q0 = decode(2)  # sum u^2

    den = np.maximum(np.sqrt(q0), EPS)
    ahat2 = q0 / (den * den)

    def dist(idx, q):
        dot = (q - q0 - q0[idx]) / 2.0
        S = ahat2 + ahat2[idx] - 2.0 * dot / (den * den[idx]) + D * EPS * EPS
        return np.sqrt(np.maximum(S, 0.0)) + EPS

    d_pos = dist(pidx, q1)
    d_neg = dist(nidx, q2)
    pos_loss = d_pos * d_pos
    neg_loss = np.maximum(MARGIN - d_neg, EPS) ** 2
    total = pos_loss.sum() + neg_loss.sum()
    return np.array(total / (2.0 * N), dtype=np.float32)
